# revision 16
# baseline (speedup 1.0000x reference)
"""Trainium2 Bass kernel for nn_CFGEmbeder (masked attention pooling).

Reference computation (per batch sample, B=128, N=512 nodes, H=512):
    h      = tanh(code_feat @ W_sa + b_sa)         [N, H]
    scores = h @ w_sc (+ b_sc)                      [N]
    attn   = softmax(scores masked by node_mask)    [N]
    out    = tanh(attn @ code_feat)                 [H]

Sharding: pure data parallel over batch; 16 samples per NeuronCore x 8 cores.
b_sc is dropped: softmax is shift invariant, so it cannot affect the output.

Per-core device algorithm (matmuls in fp16 with fp32 PSUM accumulation):
  - x, W_sa, w_sc are cast to fp16 host-side; x loads as ONE HWDGE DMA.
  - xT (partition=feature) produced by 4 HWDGE xbar transpose DMAs (nc.sync).
    The walrus DMA_DIRECT2D_XPOSE descriptor has a single sync-wait slot, so
    the DMA graph is arranged to give each transpose at most one wait:
    one producer DMA for all of x, transposes alone on the SP ring, the
    other transfers on the ACT ring (nc.scalar) or SWDGE (nc.gpsimd), and
    few enough HWDGE DMAs that Tile's 8 completion lanes are never reused.
  - mm1: hT[m] = sum_k W[k,m].T @ xT[k]  -> PSUM; tanh+bias fused on ScalarE.
  - scoresT columns per (n-chunk, sample): lhsT=tanh_hT tile, rhs=w_sc col.
  - scoresT -> scores [16, 512] via PE transposes (identity), then one
    batched masked softmax: masked = (scores + 1000) * mask (shift trick),
    exp with accum_out giving row sums in the same ScalarE op.
  - pooledT columns per (h-chunk, sample): lhsT=x-natural tile, rhs=attnT.
  - pooledT -> pooled [16, 512] via PE transposes, fused tanh, one store.
"""

from contextlib import ExitStack

import numpy as np

import concourse.bass as bass
import concourse.bacc as bacc
import concourse.mybir as mybir
import concourse.tile as tile
from concourse.bass_utils import run_bass_kernel_spmd

F16 = mybir.dt.float16
F32 = mybir.dt.float32
I32 = mybir.dt.int32

B, N, H = 128, 512, 512
NCORES = 8
S = B // NCORES          # samples per core
KC = H // 128            # 4 contraction chunks
MC = H // 128            # 4 output-feature chunks
CC = N // 128            # 4 node chunks
GS = 4                   # samples per transpose DMA
NG = S // GS
MASK_SHIFT = 1000.0      # (scores + SHIFT) * mask; softmax is shift invariant


def build_program():
    nc = bacc.Bacc(trn_type="TRN2", target_bir_lowering=False,
                   num_devices=NCORES)

    x_h = nc.dram_tensor("x", [S, N, H], F16, kind="ExternalInput")
    mask_h = nc.dram_tensor("mask", [S, N], I32, kind="ExternalInput")
    wsa_h = nc.dram_tensor("w_sa", [H, H], F16, kind="ExternalInput")
    bsa_h = nc.dram_tensor("b_sa", [H], F32, kind="ExternalInput")
    wsc_h = nc.dram_tensor("w_sc", [H], F16, kind="ExternalInput")
    id_h = nc.dram_tensor("ident", [128, 128], F32, kind="ExternalInput")
    out_h = nc.dram_tensor("out", [S, H], F32, kind="ExternalOutput")

    x = x_h.ap()
    Tanh = mybir.ActivationFunctionType.Tanh
    Exp = mybir.ActivationFunctionType.Exp
    Alu = mybir.AluOpType

    with tile.TileContext(nc) as tc, ExitStack() as ctx:
        const = ctx.enter_context(tc.tile_pool(name="const", bufs=1))
        xnat_p = ctx.enter_context(tc.tile_pool(name="xnat", bufs=1))
        xt_p = ctx.enter_context(tc.tile_pool(name="xt", bufs=1))
        th_p = ctx.enter_context(tc.tile_pool(name="th", bufs=2))
        sm_p = ctx.enter_context(tc.tile_pool(name="sm", bufs=1))
        ph_p = ctx.enter_context(tc.tile_pool(name="ph", bufs=5, space="PSUM"))
        pcol_p = ctx.enter_context(tc.tile_pool(name="pcol", bufs=1, space="PSUM"))
        prow_p = ctx.enter_context(tc.tile_pool(name="prow", bufs=1, space="PSUM"))

        # ---- constants ----
        # Everything loads on the ACT HWDGE ring (nc.scalar): any SWDGE
        # traffic makes the xbar transposes wait on it (deadlock guard),
        # and the XPOSE descriptor only has one wait slot.
        Wf = const.tile([128, KC, H], F16, name="Wf")
        nc.scalar.dma_start(Wf, wsa_h.ap().rearrange("(k p) h -> p k h", p=128))
        wsc = const.tile([128, MC], F16, name="wsc")
        nc.scalar.dma_start(wsc, wsc_h.ap().rearrange("(c p) -> p c", p=128))
        bsa = const.tile([128, MC], F32, name="bsa")
        nc.scalar.dma_start(bsa, bsa_h.ap().rearrange("(c p) -> p c", p=128))
        idf = const.tile([128, 128], F32, name="idf")
        nc.scalar.dma_start(idf, id_h.ap())
        maski = const.tile([S, N], I32, name="maski")
        nc.scalar.dma_start(maski, mask_h.ap())
        maskf = const.tile([S, N], F32, name="maskf")
        nc.vector.tensor_copy(maskf, maski)

        # ---- x: one big natural-layout load + 4 xbar transposes ----
        xnat = xnat_p.tile([128, S, CC, H], F16, name="xnat")
        nc.scalar.dma_start(xnat, x.rearrange("s (c p) h -> p s c h", p=128))
        # xt layout: [128(u=feat%128), s, c(node chunk), k(feat chunk), v]
        xt = xt_p.tile([128, S, CC, KC, 128], F16, name="xt")
        for g in range(NG):
            sl = slice(g * GS, (g + 1) * GS)
            # out[u, (s c k), v] = in[v, (s c)*512 + k*128 + u]
            nc.sync.dma_start(xt[:, sl], xnat[:, sl], transpose=True)

        psum_scT = pcol_p.tile([128, CC * S], F32, name="scT")
        psum_pT = pcol_p.tile([128, MC * S], F32, name="pT")

        # ---- phase A: per-sample matmul1 + tanh + scoresT ----
        for s in range(S):
            th = th_p.tile([128, MC, N], F16, name="th")
            for m in range(MC):
                ph = ph_p.tile([128, N], F32, name="ph")
                for k in range(KC):
                    nc.tensor.matmul(
                        ph,
                        lhsT=Wf[:, k, m * 128:(m + 1) * 128],
                        rhs=xt[:, s, :, k, :],
                        start=(k == 0),
                        stop=(k == KC - 1),
                    )
                nc.scalar.activation(th[:, m, :], ph, Tanh,
                                     bias=bsa[:, m:m + 1])

            for c in range(CC):
                col = c * S + s
                for m in range(MC):
                    nc.tensor.matmul(
                        psum_scT[:, col:col + 1],
                        lhsT=th[:, m, c * 128:(c + 1) * 128],
                        rhs=wsc[:, m:m + 1],
                        start=(m == 0),
                        stop=(m == MC - 1),
                    )

        # ---- phase B: softmax over nodes for all samples at once ----
        scT_sb = sm_p.tile([128, CC * S], F32, name="scT_sb")
        nc.vector.tensor_copy(scT_sb, psum_scT)

        psum_sc = prow_p.tile([S, N], F32, name="prow")
        for c in range(CC):
            nc.tensor.transpose(psum_sc[:, c * 128:(c + 1) * 128],
                                scT_sb[:, c * S:(c + 1) * S], idf)

        masked = sm_p.tile([S, N], F32, name="masked")
        nc.vector.scalar_tensor_tensor(masked, psum_sc, MASK_SHIFT, maskf,
                                       op0=Alu.add, op1=Alu.mult)
        nmax = sm_p.tile([S, 1], F32, name="nmax")
        nc.vector.tensor_reduce(nmax, masked, axis=mybir.AxisListType.X,
                                op=Alu.max, negate=True)
        ex = sm_p.tile([S, N], F32, name="ex")
        esum = sm_p.tile([S, 1], F32, name="esum")
        nc.scalar.activation(ex, masked, Exp, bias=nmax, accum_out=esum)
        rinv = sm_p.tile([S, 1], F32, name="rinv")
        nc.vector.reciprocal(rinv, esum)
        attn = sm_p.tile([S, N], F32, name="attn")
        nc.vector.tensor_scalar_mul(attn, ex, rinv)

        psum_aT = prow_p.tile([128, CC * S], F32, name="prow")
        for c in range(CC):
            nc.tensor.transpose(psum_aT[:, c * S:(c + 1) * S],
                                attn[:, c * 128:(c + 1) * 128],
                                idf[0:S, 0:S])
        attnT = sm_p.tile([128, CC * S], F16, name="attnT")
        nc.vector.tensor_copy(attnT, psum_aT)

        # ---- phase C: attention pooling ----
        for s in range(S):
            for j in range(MC):
                col = j * S + s
                for c in range(CC):
                    nc.tensor.matmul(
                        psum_pT[:, col:col + 1],
                        lhsT=xnat[:, s, c, j * 128:(j + 1) * 128],
                        rhs=attnT[:, c * S + s:c * S + s + 1],
                        start=(c == 0),
                        stop=(c == CC - 1),
                    )

        pT_sb = sm_p.tile([128, MC * S], F32, name="pT_sb")
        nc.vector.tensor_copy(pT_sb, psum_pT)
        psum_pool = prow_p.tile([S, H], F32, name="prow")
        for j in range(MC):
            nc.tensor.transpose(psum_pool[:, j * 128:(j + 1) * 128],
                                pT_sb[:, j * S:(j + 1) * S], idf)
        out_sb = sm_p.tile([S, H], F32, name="out_sb")
        nc.scalar.activation(out_sb, psum_pool, Tanh)
        nc.scalar.dma_start(out_h.ap(), out_sb)

    nc.finalize()
    return nc


_CACHE = {}


def _get_nc():
    if "nc" not in _CACHE:
        _CACHE["nc"] = build_program()
    return _CACHE["nc"]


def make_in_maps(code_feat, node_mask, W_sa, b_sa, w_sc):
    ident = np.eye(128, dtype=np.float32)
    x16 = np.asarray(code_feat, dtype=np.float16)
    w16 = np.asarray(W_sa, dtype=np.float16)
    wsc16 = np.asarray(w_sc, dtype=np.float16)
    in_maps = []
    for i in range(NCORES):
        sl = slice(i * S, (i + 1) * S)
        in_maps.append({
            "x": np.ascontiguousarray(x16[sl]),
            "mask": np.ascontiguousarray(node_mask[sl], dtype=np.int32),
            "w_sa": w16,
            "b_sa": np.asarray(b_sa, dtype=np.float32),
            "w_sc": wsc16,
            "ident": ident,
        })
    return in_maps


def kernel(code_feat, node_mask, W_sa, b_sa, w_sc, b_sc=None, **_ignored):
    code_feat = np.asarray(code_feat)
    node_mask = np.asarray(node_mask)
    nc = _get_nc()
    in_maps = make_in_maps(code_feat, node_mask, W_sa, b_sa, w_sc)
    res = run_bass_kernel_spmd(nc, in_maps, list(range(NCORES)))
    out = np.concatenate([r["out"] for r in res.results], axis=0)
    return out.astype(np.float32)
